# revision 32
# baseline (speedup 1.0000x reference)
"""Point-Transformer attention block on 8 Trainium2 NeuronCores.

Shards the points axis N across 8 cores (all ops are pointwise in N).
Per core: channels on SBUF partitions, pixels on the free dim.  Pixels are
host-reordered k-major within each 256-point tile, so every (n,k) view
becomes a contiguous slice.  All matmuls use bf16 operands (1 cycle/row)
accumulating in fp32 PSUM.

Fusions vs the reference:
  - x3/d in one psum: stationary [w3; -w2] + [pw2; pw2] -> psum rows
    0:64 = x3_pre (+ptf), 64:128 = d = ptf - x2_pre; one scalar-engine
    activation adds [b3; 0] and materializes both halves as bf16.
  - ppfs host-packed [16, npix/2] (chunk pairs stacked on partitions) so
    one blockdiag(pw1) matmul computes two chunks of r at once.
  - x1 / xfs / cw1 folded:  h = relu( sum_k cw1_k^T @ d_k
                                      + (cw1s @ w1) @ x + cw1s @ (b1 - b2) )
  - softmax denominators applied at the END (U * bcast(1/sum)) so the
    reciprocal never blocks the tensor engine; exp -> 0/1-matmul group
    sums -> fast approx reciprocal -> f32r 0/1-matmul broadcast.
  - k-sum of q: pairwise adds on DVE then 8 accumulating identity matmuls
  - identity skip + bout folded into the final DVE scalar_tensor_tensor.
Emission is software-pipelined: S1(i) then an interleaved zone running
tile i-1's attention-apply against tile i's h/e computation, so neither
the tensor engine nor DVE ever drains.
"""

import numpy as np

B, CIN, N, K = 4, 64, 16384, 16
MID, OUT, PT, SHARE = 64, 128, 8, 8
G = MID // SHARE  # 8 softmax groups
NCORES = 8
NS = N // NCORES          # points per core per batch (2048)
P = 512                   # points per tile
PK = P * K                # pixels per tile (4096)
FD = 512                  # pixel chunk (one matmul's moving width)
NCH = PK // FD            # chunks per tile (8)


def _build_consts(w0, b0, w1, b1, w2, b2, w3, b3, pw1, pw2, cw1, cw2, cb2,
                  wout, bout):
    import ml_dtypes
    f32 = np.float32
    bf = ml_dtypes.bfloat16
    c = {}
    c["w0t"] = np.ascontiguousarray(w0.T).astype(bf)                # [64,128]
    z64 = np.zeros((OUT, MID), np.float32)
    c["w3lo"] = np.concatenate([w3.T, z64], axis=1).astype(bf)      # [128,128]
    c["w3hi"] = np.concatenate([z64, w3.T], axis=1).astype(bf)      # [128,128]
    c["mw2lo"] = np.concatenate([-w2.T, z64], axis=1).astype(bf)    # [128,128]
    c["mw2hi"] = np.concatenate([z64, -w2.T], axis=1).astype(bf)    # [128,128]
    zpw = np.zeros_like(pw2.T)
    c["pw2blk"] = np.block([[pw2.T, zpw], [zpw, pw2.T]]).astype(bf)  # [128,128]
    pw1t = np.ascontiguousarray(pw1.T)                              # [8,64]
    pw1t2 = np.zeros((2 * PT, OUT), np.float32)
    pw1t2[0:PT, 0:MID] = pw1t
    pw1t2[PT:2 * PT, MID:OUT] = pw1t
    c["pw1t2"] = pw1t2.astype(bf)                                   # [16,128]
    cw1r = cw1.reshape(G, MID, K)
    cw1s = cw1r.sum(-1)                                             # [8,64]
    c["cat"] = np.ascontiguousarray((cw1s @ w1).T).astype(bf)       # [128,8]
    gkp = np.zeros((OUT, (K // 2) * G), np.float32)
    for j in range(K // 2):
        gkp[0:MID, j * G:(j + 1) * G] = cw1r[:, :, 2 * j].T
        gkp[MID:OUT, j * G:(j + 1) * G] = cw1r[:, :, 2 * j + 1].T
    c["gkp"] = gkp.astype(bf)                                       # [128,64]
    c["hb"] = np.ascontiguousarray((cw1s @ (b1 - b2))[:, None], f32)  # [8,1]
    c["cw2t"] = np.ascontiguousarray(cw2.T).astype(bf)              # [8,128]
    c["cb2"] = np.ascontiguousarray(cb2[:, None], f32)              # [128,1]
    bsum = np.zeros((OUT, G), f32)
    for g in range(G):
        bsum[g * K:(g + 1) * K, g] = 1.0
    c["bsum"] = bsum.astype(bf)                                     # [128,8]
    bsum64 = np.zeros((G, MID), f32)
    for m in range(MID):
        bsum64[m % G, m] = 1.0
    c["bsum64"] = bsum64.astype(bf)                                 # [8,64]
    bk2 = np.zeros((OUT, (K // 2) * OUT), f32)
    for j in range(K // 2):
        for m in range(OUT):
            k = 2 * j + (m >= MID)
            bk2[(m % G) * K + k, j * OUT + m] = 1.0
    c["bk2"] = bk2.astype(bf)                                       # [128,1024]
    c["woutt"] = np.ascontiguousarray(wout.T).astype(bf)            # [64,128]
    c["eye2"] = np.concatenate(
        [np.eye(MID, dtype=f32)] * 2, axis=0).astype(bf)            # [128,64]
    c["b0"] = np.ascontiguousarray(b0[:, None], f32)                # [128,1]
    c["b3b"] = np.concatenate([b3, b3])[:, None].astype(f32)        # [128,1]
    c["bout"] = np.ascontiguousarray(bout[:, None], f32)            # [128,1]
    return c


def _build_program(ns_pts):
    import concourse.bass as bass
    import concourse.tile as tile
    from concourse import mybir
    from contextlib import ExitStack

    f32 = mybir.dt.float32
    f32r = mybir.dt.float32r
    bf16 = mybir.dt.bfloat16
    AF = mybir.ActivationFunctionType
    ALU = mybir.AluOpType
    nt = ns_pts // P

    nc = bass.Bass()
    feats_d = nc.declare_dram_parameter("feats", [B, CIN, ns_pts * K], bf16, isOutput=False)
    ppfs_d = nc.declare_dram_parameter("ppfs", [B, 2 * PT, ns_pts * K // 2], bf16, isOutput=False)
    cshape = dict(
        w0t=([CIN, OUT], bf16), w3lo=([OUT, OUT], bf16), w3hi=([OUT, OUT], bf16),
        mw2lo=([OUT, OUT], bf16), mw2hi=([OUT, OUT], bf16), pw2blk=([OUT, OUT], bf16),
        pw1t2=([2 * PT, OUT], bf16), cat=([OUT, G], bf16),
        gkp=([OUT, (K // 2) * G], bf16),
        hb=([G, 1], f32), cw2t=([G, OUT], bf16), cb2=([OUT, 1], f32),
        bsum=([OUT, G], bf16), bsum64=([G, MID], bf16),
        bk2=([OUT, (K // 2) * OUT], bf16),
        woutt=([MID, OUT], bf16), eye2=([OUT, MID], bf16),
        b0=([OUT, 1], f32), b3b=([OUT, 1], f32), bout=([OUT, 1], f32),
    )
    cdram = {k: nc.declare_dram_parameter(k, v[0], v[1], isOutput=False)
             for k, v in cshape.items()}
    out_d = nc.declare_dram_parameter("out", [B, OUT, ns_pts], f32, isOutput=True)

    ntot = B * nt

    with tile.TileContext(nc) as tc, ExitStack() as ctx:
        consts = ctx.enter_context(tc.tile_pool(name="consts", bufs=1))
        ct = {k: consts.tile_from(v[:], name=k) for k, v in cdram.items()}

        io = ctx.enter_context(tc.tile_pool(name="io", bufs=2))
        sb = ctx.enter_context(tc.tile_pool(name="sb", bufs=2))
        small = ctx.enter_context(tc.tile_pool(name="small", bufs=2))
        pix = ctx.enter_context(tc.tile_pool(name="pix", bufs=2, space="PSUM"))
        ptp = ctx.enter_context(tc.tile_pool(name="ptp", bufs=2, space="PSUM"))
        wfp = ctx.enter_context(tc.tile_pool(name="wfp", bufs=2, space="PSUM"))

        st = {}  # per-tile live handles

        def emit_dma(i):
            b, t = divmod(i, nt)
            px = slice(t * PK, (t + 1) * PK)
            px2 = slice(t * PK // 2, (t + 1) * PK // 2)
            ft = io.tile([CIN, PK], bf16, tag="ft")
            for w in range(4):
                rs_ = slice(w * CIN // 4, (w + 1) * CIN // 4)
                nc.sync.dma_start(ft[rs_, :], feats_d[b, rs_, px])
            pf = io.tile([2 * PT, PK // 2], bf16, tag="pf")
            nc.sync.dma_start(pf[0:PT, :], ppfs_d[b, 0:PT, px2])
            nc.sync.dma_start(pf[PT:2 * PT, :], ppfs_d[b, PT:2 * PT, px2])
            st[i] = dict(ft=ft, pf=pf)

        def emit_s1a(i):
            s = st[i]
            ft, pf = s["ft"], s["pf"]
            # xn = relu(w0 @ feats + b0)            [128, PK] bf16
            xn = sb.tile([OUT, PK], bf16, tag="xn")
            s["xn"] = xn
            for u in range(NCH // 2):
                ps = pix.tile([OUT, 2 * FD], f32, tag="pix")
                for h in range(2):
                    cs = slice((2 * u + h) * FD, (2 * u + h + 1) * FD)
                    nc.tensor.matmul(out=ps[:, h * FD:(h + 1) * FD],
                                     lhsT=ct["w0t"][:], rhs=ft[:, cs],
                                     start=True, stop=True)
                if u < 5:
                    nc.scalar.activation(xn[:, 2 * u * FD:(2 * u + 2) * FD],
                                         ps[:], AF.Relu, bias=ct["b0"][:])
                else:
                    nc.vector.tensor_scalar(
                        xn[:, 2 * u * FD:(2 * u + 2) * FD], ps[:],
                        ct["b0"][:], 0.0, op0=ALU.add, op1=ALU.max)
            # r = relu(pw1 @ ppfs), chunk c on rows 64*(c%2), cols (c//2)*FD
            r = sb.tile([OUT, PK // 2], bf16, tag="r")
            s["r"] = r
            for u in range(NCH // 4):
                ps = pix.tile([OUT, 2 * FD], f32, tag="pix")
                for h in range(2):
                    js = slice((2 * u + h) * FD, (2 * u + h + 1) * FD)
                    nc.tensor.matmul(out=ps[:, h * FD:(h + 1) * FD],
                                     lhsT=ct["pw1t2"][:], rhs=pf[:, js],
                                     start=True, stop=True)
                if u < NCH // 4 - 2:
                    nc.scalar.activation(r[:, 2 * u * FD:(2 * u + 2) * FD],
                                         ps[:], AF.Relu)
                elif u < NCH // 4 - 1:
                    nc.vector.tensor_scalar_max(
                        r[:, 2 * u * FD:(2 * u + 2) * FD], ps[:], 0.0)
                else:
                    s["rps_last"] = (ps, 2 * u * FD)

        def emit_s1b(i):
            s = st[i]
            xn, r = s["xn"], s["r"]
            ps_last, off = s.pop("rps_last")
            nc.vector.tensor_scalar_max(r[:, off:off + 2 * FD], ps_last[:], 0.0)
            # pair-stacked psums: [x3_2j ; x3_2j+1] and [d_2j ; d_2j+1]
            x3p = sb.tile([OUT, PK // 2], bf16, tag="x3p")
            dp = sb.tile([OUT, PK // 2], bf16, tag="dp")
            nsc = 0
            for v in range(K // 4):
                for which in range(2):  # 0: x3 pairs, 1: d pairs
                    ps = pix.tile([OUT, 2 * FD], f32, tag="pix")
                    for w in range(2):
                        j = 2 * v + w
                        hs = slice(w * FD, (w + 1) * FD)
                        lo = "w3lo" if which == 0 else "mw2lo"
                        hi = "w3hi" if which == 0 else "mw2hi"
                        nc.tensor.matmul(out=ps[:, hs], lhsT=ct[lo][:],
                                         rhs=xn[:, (2 * j) * FD:(2 * j + 1) * FD],
                                         start=True, stop=False)
                        nc.tensor.matmul(out=ps[:, hs], lhsT=ct[hi][:],
                                         rhs=xn[:, (2 * j + 1) * FD:(2 * j + 2) * FD],
                                         start=False, stop=False)
                        nc.tensor.matmul(out=ps[:, hs], lhsT=ct["pw2blk"][:],
                                         rhs=r[:, j * FD:(j + 1) * FD],
                                         start=False, stop=True)
                    dst = (x3p if which == 0 else dp)[:, 2 * v * FD:(2 * v + 2) * FD]
                    if which == 0:
                        if nsc < 3:
                            nc.scalar.activation(dst, ps[:], AF.Identity,
                                                 bias=ct["b3b"][:])
                        else:
                            nc.vector.tensor_scalar_add(dst, ps[:], ct["b3b"][:])
                        nsc += 1
                    else:
                        if v < 2:
                            nc.scalar.activation(dst, ps[:], AF.Identity)
                        else:
                            nc.vector.tensor_copy(dst, ps[:])
            s["x3p"], s["dp"] = x3p, dp

        def emit_qpart(ip, jlo, jhi):
            sp = st[ip]
            if "q" not in sp:
                sp["q"] = sb.tile([OUT, PK // 2], bf16, tag="q", name="q")
            q = sp["q"]
            for j in range(jlo, jhi):
                wf = wfp.tile([OUT, P], f32, tag="wf")
                nc.tensor.matmul(out=wf[:],
                                 lhsT=ct["bk2"][:, j * OUT:(j + 1) * OUT],
                                 rhs=sp["e"][:], start=True, stop=True)
                nc.vector.tensor_mul(q[:, j * P:(j + 1) * P],
                                     sp["x3p"][:, j * P:(j + 1) * P], wf[:])

        def emit_e1(ic):
            sc = st[ic]
            # h = relu(sum_k cw1_k^T @ d_k + Ca @ x + hb)   [8, P]
            hps = ptp.tile([G, P], f32, tag="pt")
            for j in range(K // 2):
                nc.tensor.matmul(out=hps[:], lhsT=ct["gkp"][:, j * G:(j + 1) * G],
                                 rhs=sc["dp"][:, j * P:(j + 1) * P],
                                 start=(j == 0), stop=False)
            nc.tensor.matmul(out=hps[:], lhsT=ct["cat"][:],
                             rhs=sc["xn"][:, 0:P], start=False, stop=True)
            h = small.tile([G, P], bf16, tag="h")
            nc.scalar.activation(h[:], hps[:], AF.Relu, bias=ct["hb"][:])
            sc["h"] = h

        def emit_t1(ip):
            # U = sum_j q2_j; o = relu(U) * bcast(rs)
            sp = st[ip]
            q = sp["q"]
            ups = ptp.tile([MID, P], f32, tag="pt")
            for j in range(K // 2):
                nc.tensor.matmul(out=ups[:], lhsT=ct["eye2"][:],
                                 rhs=q[:, j * P:(j + 1) * P],
                                 start=(j == 0), stop=(j == K // 2 - 1))
            rsb = ptp.tile([MID, P], f32, tag="pt")
            nc.tensor.matmul(out=rsb[:], lhsT=ct["bsum64"][:],
                             rhs=sp["rs"][:], start=True, stop=True)
            un = small.tile([MID, P], bf16, tag="un")
            nc.scalar.activation(un[:], ups[:], AF.Relu)
            o = small.tile([MID, P], bf16, tag="o")
            nc.vector.tensor_mul(o[:], un[:], rsb[:])
            sp["o"] = o

        def emit_e2(ic):
            sc = st[ic]
            # e = exp(cw2 @ h + cb2); rs = exp(-ln(bsum @ e))
            eps = ptp.tile([OUT, P], f32, tag="pt")
            nc.tensor.matmul(out=eps[:], lhsT=ct["cw2t"][:], rhs=sc["h"][:],
                             start=True, stop=True)
            e = small.tile([OUT, P], bf16, tag="e")
            nc.scalar.activation(e[:], eps[:], AF.Exp, bias=ct["cb2"][:])
            sps = ptp.tile([G, P], f32, tag="pt")
            nc.tensor.matmul(out=sps[:], lhsT=ct["bsum"][:], rhs=e[:],
                             start=True, stop=True)
            lns = small.tile([G, P], f32, tag="lns")
            nc.scalar.activation(lns[:], sps[:], AF.Ln)
            rs = small.tile([G, P], bf16, tag="rs")
            nc.scalar.activation(rs[:], lns[:], AF.Exp, scale=-1.0)
            sc["e"], sc["rs"] = e, rs

        def emit_t2(ip):
            # out = wout @ o + x + bout
            sp = st[ip]
            b, t = divmod(ip, nt)
            ops_ = ptp.tile([OUT, P], f32, tag="pt")
            nc.tensor.matmul(out=ops_[:], lhsT=ct["woutt"][:], rhs=sp["o"][:],
                             start=True, stop=True)
            res = small.tile([OUT, P], f32, tag="res")
            nc.vector.scalar_tensor_tensor(
                res[:], ops_[:], ct["bout"][:], sp["xn"][:, 0:P],
                op0=ALU.add, op1=ALU.add)
            nc.sync.dma_start(out_d[b, :, t * P:(t + 1) * P], res[:])
            del st[ip]

        emit_dma(0)
        for i in range(ntot):
            if i + 1 < ntot:
                emit_dma(i + 1)
            emit_s1a(i)
            if i >= 1:
                emit_qpart(i - 1, 0, K // 4)
            emit_s1b(i)
            if i >= 1:
                emit_qpart(i - 1, K // 4, K // 2)
                emit_t1(i - 1)
                emit_t2(i - 1)
            emit_e1(i)
            emit_e2(i)
        emit_qpart(ntot - 1, 0, K // 2)
        emit_t1(ntot - 1)
        emit_t2(ntot - 1)

    return nc


def _legalize_waits(nc):
    """This toolchain's walrus rejects >1 sync-wait per instruction; hoist
    extra waits onto same-engine event-semaphore instructions just before."""
    from concourse import mybir

    n_split = 0
    for fn in nc.m.functions:
        for bb in fn.blocks:
            insts = bb.instructions
            new_list = []
            for inst in insts:
                si = inst.sync_info
                if si is not None and si.on_wait is not None and len(si.on_wait) > 1:
                    waits = list(si.on_wait)
                    for j, w in enumerate(waits[:-1]):
                        ev = mybir.InstEventSemaphore(
                            name=f"{inst.name}-lw{j}", ins=[], outs=[])
                        ev.engine = inst.engine
                        ev.sync_info = mybir.SyncInfo(on_wait=[w], on_update=[])
                        new_list.append(ev)
                        n_split += 1
                    inst.sync_info = mybir.SyncInfo(
                        on_wait=[waits[-1]], on_update=list(si.on_update))
                new_list.append(inst)
            if len(new_list) != len(insts):
                insts[:] = new_list
    return n_split


LAST_RESULTS = None


def kernel(sm_feats, sm_ppfs, w0, b0, w1, b1, w2, b2, w3, b3,
           pw1, pw2, cw1, cw2, cb2, wout, bout):
    global LAST_RESULTS
    import ml_dtypes
    from concourse.bass_utils import run_bass_kernel_spmd

    bf = ml_dtypes.bfloat16
    consts = _build_consts(w0, b0, w1, b1, w2, b2, w3, b3, pw1, pw2,
                           cw1, cw2, cb2, wout, bout)
    nc = _build_program(NS)
    _legalize_waits(nc)

    feats_bf = np.ascontiguousarray(sm_feats).astype(bf)
    ppfs_bf = np.ascontiguousarray(sm_ppfs).astype(bf)
    npix = NS * K

    in_maps = []
    for i in range(NCORES):
        sl = slice(i * NS, (i + 1) * NS)
        m = dict(consts)
        # k-major pixel order within each 256-point tile
        f = feats_bf[:, :, sl, :].reshape(B, CIN, NS // P, P, K)
        m["feats"] = np.ascontiguousarray(
            f.transpose(0, 1, 2, 4, 3)).reshape(B, CIN, npix)
        p = ppfs_bf[:, :, sl, :].reshape(B, PT, NS // P, P, K)
        pp = np.ascontiguousarray(
            p.transpose(0, 1, 2, 4, 3)).reshape(B, PT, npix)
        # pack chunk pairs on partitions: [b, h*8+ch, j*512+s] = pp[b, ch, (2j+h)*512+s]
        v = pp.reshape(B, PT, npix // 1024, 2, FD)
        m["ppfs"] = np.ascontiguousarray(
            v.transpose(0, 3, 1, 2, 4)).reshape(B, 2 * PT, npix // 2)
        in_maps.append(m)

    res = run_bass_kernel_spmd(nc, in_maps, list(range(NCORES)))
    LAST_RESULTS = res
    shards = [res.results[i]["out"] for i in range(NCORES)]
    return np.concatenate(shards, axis=2)


# revision 33
# speedup vs baseline: 1.2594x; 1.2594x over previous
"""Point-Transformer attention block on 8 Trainium2 NeuronCores.

Shards the points axis N across 8 cores (all ops are pointwise in N).
Per core: channels on SBUF partitions, pixels on the free dim.  Pixels are
host-reordered k-major within each 256-point tile, so every (n,k) view
becomes a contiguous slice.  All matmuls use bf16 operands (1 cycle/row)
accumulating in fp32 PSUM.

Fusions vs the reference:
  - x3/d in one psum: stationary [w3; -w2] + [pw2; pw2] -> psum rows
    0:64 = x3_pre (+ptf), 64:128 = d = ptf - x2_pre; one scalar-engine
    activation adds [b3; 0] and materializes both halves as bf16.
  - ppfs host-packed [16, npix/2] (chunk pairs stacked on partitions) so
    one blockdiag(pw1) matmul computes two chunks of r at once.
  - x1 / xfs / cw1 folded:  h = relu( sum_k cw1_k^T @ d_k
                                      + (cw1s @ w1) @ x + cw1s @ (b1 - b2) )
  - softmax denominators applied at the END (U * bcast(1/sum)) so the
    reciprocal never blocks the tensor engine; exp -> 0/1-matmul group
    sums -> fast approx reciprocal -> f32r 0/1-matmul broadcast.
  - k-sum of q: pairwise adds on DVE then 8 accumulating identity matmuls
  - identity skip + bout folded into the final DVE scalar_tensor_tensor.
Emission is software-pipelined: S1(i) then an interleaved zone running
tile i-1's attention-apply against tile i's h/e computation, so neither
the tensor engine nor DVE ever drains.
"""

import numpy as np

B, CIN, N, K = 4, 64, 16384, 16
MID, OUT, PT, SHARE = 64, 128, 8, 8
G = MID // SHARE  # 8 softmax groups
NCORES = 8
NS = N // NCORES          # points per core per batch (2048)
P = 512                   # points per tile
PK = P * K                # pixels per tile (4096)
FD = 512                  # pixel chunk (one matmul's moving width)
NCH = PK // FD            # chunks per tile (8)


def _build_consts(w0, b0, w1, b1, w2, b2, w3, b3, pw1, pw2, cw1, cw2, cb2,
                  wout, bout):
    import ml_dtypes
    f32 = np.float32
    bf = ml_dtypes.bfloat16
    c = {}
    c["w0t"] = np.ascontiguousarray(w0.T).astype(bf)                # [64,128]
    z64 = np.zeros((OUT, MID), np.float32)
    c["w3lo"] = np.concatenate([w3.T, z64], axis=1).astype(bf)      # [128,128]
    c["w3hi"] = np.concatenate([z64, w3.T], axis=1).astype(bf)      # [128,128]
    c["mw2lo"] = np.concatenate([-w2.T, z64], axis=1).astype(bf)    # [128,128]
    c["mw2hi"] = np.concatenate([z64, -w2.T], axis=1).astype(bf)    # [128,128]
    zpw = np.zeros_like(pw2.T)
    c["pw2blk"] = np.block([[pw2.T, zpw], [zpw, pw2.T]]).astype(bf)  # [128,128]
    pw1t = np.ascontiguousarray(pw1.T)                              # [8,64]
    pw1t2 = np.zeros((2 * PT, OUT), np.float32)
    pw1t2[0:PT, 0:MID] = pw1t
    pw1t2[PT:2 * PT, MID:OUT] = pw1t
    c["pw1t2"] = pw1t2.astype(bf)                                   # [16,128]
    cw1r = cw1.reshape(G, MID, K)
    cw1s = cw1r.sum(-1)                                             # [8,64]
    c["cat"] = np.ascontiguousarray((cw1s @ w1).T).astype(bf)       # [128,8]
    gkp = np.zeros((OUT, (K // 2) * G), np.float32)
    for j in range(K // 2):
        gkp[0:MID, j * G:(j + 1) * G] = cw1r[:, :, 2 * j].T
        gkp[MID:OUT, j * G:(j + 1) * G] = cw1r[:, :, 2 * j + 1].T
    c["gkp"] = gkp.astype(bf)                                       # [128,64]
    c["hb"] = np.ascontiguousarray((cw1s @ (b1 - b2))[:, None], f32)  # [8,1]
    c["cw2t"] = np.ascontiguousarray(cw2.T).astype(bf)              # [8,128]
    c["cb2"] = np.ascontiguousarray(cb2[:, None], f32)              # [128,1]
    bsum = np.zeros((OUT, G), f32)
    for g in range(G):
        bsum[g * K:(g + 1) * K, g] = 1.0
    c["bsum"] = bsum.astype(bf)                                     # [128,8]
    bsum64 = np.zeros((G, MID), f32)
    for m in range(MID):
        bsum64[m % G, m] = 1.0
    c["bsum64"] = bsum64.astype(bf)                                 # [8,64]
    bk2 = np.zeros((OUT, (K // 2) * OUT), f32)
    for j in range(K // 2):
        for m in range(OUT):
            k = 2 * j + (m >= MID)
            bk2[(m % G) * K + k, j * OUT + m] = 1.0
    c["bk2"] = bk2.astype(bf)                                       # [128,1024]
    c["woutt"] = np.ascontiguousarray(wout.T).astype(bf)            # [64,128]
    c["eye2"] = np.concatenate(
        [np.eye(MID, dtype=f32)] * 2, axis=0).astype(bf)            # [128,64]
    c["b0"] = np.ascontiguousarray(b0[:, None], f32)                # [128,1]
    c["b3b"] = np.concatenate([b3, b3])[:, None].astype(f32)        # [128,1]
    c["bout"] = np.ascontiguousarray(bout[:, None], f32)            # [128,1]
    return c


def _build_program(ns_pts):
    import concourse.bass as bass
    import concourse.tile as tile
    from concourse import mybir
    from contextlib import ExitStack

    f32 = mybir.dt.float32
    f32r = mybir.dt.float32r
    bf16 = mybir.dt.bfloat16
    AF = mybir.ActivationFunctionType
    ALU = mybir.AluOpType
    nt = ns_pts // P

    nc = bass.Bass()
    feats_d = nc.declare_dram_parameter("feats", [B, CIN, ns_pts * K], bf16, isOutput=False)
    ppfs_d = nc.declare_dram_parameter("ppfs", [B, 2 * PT, ns_pts * K // 2], bf16, isOutput=False)
    cshape = dict(
        w0t=([CIN, OUT], bf16), w3lo=([OUT, OUT], bf16), w3hi=([OUT, OUT], bf16),
        mw2lo=([OUT, OUT], bf16), mw2hi=([OUT, OUT], bf16), pw2blk=([OUT, OUT], bf16),
        pw1t2=([2 * PT, OUT], bf16), cat=([OUT, G], bf16),
        gkp=([OUT, (K // 2) * G], bf16),
        hb=([G, 1], f32), cw2t=([G, OUT], bf16), cb2=([OUT, 1], f32),
        bsum=([OUT, G], bf16), bsum64=([G, MID], bf16),
        bk2=([OUT, (K // 2) * OUT], bf16),
        woutt=([MID, OUT], bf16), eye2=([OUT, MID], bf16),
        b0=([OUT, 1], f32), b3b=([OUT, 1], f32), bout=([OUT, 1], f32),
    )
    cdram = {k: nc.declare_dram_parameter(k, v[0], v[1], isOutput=False)
             for k, v in cshape.items()}
    out_d = nc.declare_dram_parameter("out", [B, OUT, ns_pts], f32, isOutput=True)

    ntot = B * nt

    with tile.TileContext(nc) as tc, ExitStack() as ctx:
        consts = ctx.enter_context(tc.tile_pool(name="consts", bufs=1))
        ct = {k: consts.tile_from(v[:], name=k) for k, v in cdram.items()}

        io = ctx.enter_context(tc.tile_pool(name="io", bufs=2))
        sb = ctx.enter_context(tc.tile_pool(name="sb", bufs=2))
        small = ctx.enter_context(tc.tile_pool(name="small", bufs=2))
        pix = ctx.enter_context(tc.tile_pool(name="pix", bufs=2, space="PSUM"))
        ptp = ctx.enter_context(tc.tile_pool(name="ptp", bufs=2, space="PSUM"))
        wfp = ctx.enter_context(tc.tile_pool(name="wfp", bufs=2, space="PSUM"))

        st = {}  # per-tile live handles

        def emit_dma(i):
            b, t = divmod(i, nt)
            px = slice(t * PK, (t + 1) * PK)
            px2 = slice(t * PK // 2, (t + 1) * PK // 2)
            ft = io.tile([CIN, PK], bf16, tag="ft")
            nc.sync.dma_start(ft[0:CIN // 2, :], feats_d[b, 0:CIN // 2, px])
            nc.sync.dma_start(ft[CIN // 2:CIN, :], feats_d[b, CIN // 2:CIN, px])
            pf = io.tile([2 * PT, PK // 2], bf16, tag="pf")
            nc.sync.dma_start(pf[:], ppfs_d[b, :, px2])
            st[i] = dict(ft=ft, pf=pf)

        def emit_s1a(i):
            s = st[i]
            ft, pf = s["ft"], s["pf"]
            # xn = relu(w0 @ feats + b0)            [128, PK] bf16
            xn = sb.tile([OUT, PK], bf16, tag="xn")
            s["xn"] = xn
            for u in range(NCH // 2):
                ps = pix.tile([OUT, 2 * FD], f32, tag="pix")
                for h in range(2):
                    cs = slice((2 * u + h) * FD, (2 * u + h + 1) * FD)
                    nc.tensor.matmul(out=ps[:, h * FD:(h + 1) * FD],
                                     lhsT=ct["w0t"][:], rhs=ft[:, cs],
                                     start=True, stop=True)
                if u < 5:
                    nc.scalar.activation(xn[:, 2 * u * FD:(2 * u + 2) * FD],
                                         ps[:], AF.Relu, bias=ct["b0"][:])
                else:
                    nc.vector.tensor_scalar(
                        xn[:, 2 * u * FD:(2 * u + 2) * FD], ps[:],
                        ct["b0"][:], 0.0, op0=ALU.add, op1=ALU.max)
            # r = relu(pw1 @ ppfs), chunk c on rows 64*(c%2), cols (c//2)*FD
            r = sb.tile([OUT, PK // 2], bf16, tag="r")
            s["r"] = r
            for u in range(NCH // 4):
                ps = pix.tile([OUT, 2 * FD], f32, tag="pix")
                for h in range(2):
                    js = slice((2 * u + h) * FD, (2 * u + h + 1) * FD)
                    nc.tensor.matmul(out=ps[:, h * FD:(h + 1) * FD],
                                     lhsT=ct["pw1t2"][:], rhs=pf[:, js],
                                     start=True, stop=True)
                if u < NCH // 4 - 2:
                    nc.scalar.activation(r[:, 2 * u * FD:(2 * u + 2) * FD],
                                         ps[:], AF.Relu)
                elif u < NCH // 4 - 1:
                    nc.vector.tensor_scalar_max(
                        r[:, 2 * u * FD:(2 * u + 2) * FD], ps[:], 0.0)
                else:
                    s["rps_last"] = (ps, 2 * u * FD)

        def emit_s1b(i):
            s = st[i]
            xn, r = s["xn"], s["r"]
            ps_last, off = s.pop("rps_last")
            nc.vector.tensor_scalar_max(r[:, off:off + 2 * FD], ps_last[:], 0.0)
            # pair-stacked psums: [x3_2j ; x3_2j+1] and [d_2j ; d_2j+1]
            x3p = sb.tile([OUT, PK // 2], bf16, tag="x3p")
            dp = sb.tile([OUT, PK // 2], bf16, tag="dp")
            nsc = 0
            for v in range(K // 4):
                for which in range(2):  # 0: x3 pairs, 1: d pairs
                    ps = pix.tile([OUT, 2 * FD], f32, tag="pix")
                    for w in range(2):
                        j = 2 * v + w
                        hs = slice(w * FD, (w + 1) * FD)
                        lo = "w3lo" if which == 0 else "mw2lo"
                        hi = "w3hi" if which == 0 else "mw2hi"
                        nc.tensor.matmul(out=ps[:, hs], lhsT=ct[lo][:],
                                         rhs=xn[:, (2 * j) * FD:(2 * j + 1) * FD],
                                         start=True, stop=False)
                        nc.tensor.matmul(out=ps[:, hs], lhsT=ct[hi][:],
                                         rhs=xn[:, (2 * j + 1) * FD:(2 * j + 2) * FD],
                                         start=False, stop=False)
                        nc.tensor.matmul(out=ps[:, hs], lhsT=ct["pw2blk"][:],
                                         rhs=r[:, j * FD:(j + 1) * FD],
                                         start=False, stop=True)
                    dst = (x3p if which == 0 else dp)[:, 2 * v * FD:(2 * v + 2) * FD]
                    if which == 0:
                        if nsc < 3:
                            nc.scalar.activation(dst, ps[:], AF.Identity,
                                                 bias=ct["b3b"][:])
                        else:
                            nc.vector.tensor_scalar_add(dst, ps[:], ct["b3b"][:])
                        nsc += 1
                    else:
                        if v < 2:
                            nc.scalar.activation(dst, ps[:], AF.Identity)
                        else:
                            nc.vector.tensor_copy(dst, ps[:])
            s["x3p"], s["dp"] = x3p, dp

        def emit_qpart(ip, jlo, jhi):
            sp = st[ip]
            if "q" not in sp:
                sp["q"] = sb.tile([OUT, PK // 2], bf16, tag="q", name="q")
            q = sp["q"]
            for j in range(jlo, jhi):
                wf = wfp.tile([OUT, P], f32, tag="wf")
                nc.tensor.matmul(out=wf[:],
                                 lhsT=ct["bk2"][:, j * OUT:(j + 1) * OUT],
                                 rhs=sp["e"][:], start=True, stop=True)
                nc.vector.tensor_mul(q[:, j * P:(j + 1) * P],
                                     sp["x3p"][:, j * P:(j + 1) * P], wf[:])

        def emit_e1(ic):
            sc = st[ic]
            # h = relu(sum_k cw1_k^T @ d_k + Ca @ x + hb)   [8, P]
            hps = ptp.tile([G, P], f32, tag="pt")
            for j in range(K // 2):
                nc.tensor.matmul(out=hps[:], lhsT=ct["gkp"][:, j * G:(j + 1) * G],
                                 rhs=sc["dp"][:, j * P:(j + 1) * P],
                                 start=(j == 0), stop=False)
            nc.tensor.matmul(out=hps[:], lhsT=ct["cat"][:],
                             rhs=sc["xn"][:, 0:P], start=False, stop=True)
            h = small.tile([G, P], bf16, tag="h")
            nc.scalar.activation(h[:], hps[:], AF.Relu, bias=ct["hb"][:])
            sc["h"] = h

        def emit_t1(ip):
            # U = sum_j q2_j; o = relu(U) * bcast(rs)
            sp = st[ip]
            q = sp["q"]
            ups = ptp.tile([MID, P], f32, tag="pt")
            for j in range(K // 2):
                nc.tensor.matmul(out=ups[:], lhsT=ct["eye2"][:],
                                 rhs=q[:, j * P:(j + 1) * P],
                                 start=(j == 0), stop=(j == K // 2 - 1))
            rsb = ptp.tile([MID, P], f32, tag="pt")
            nc.tensor.matmul(out=rsb[:], lhsT=ct["bsum64"][:],
                             rhs=sp["rs"][:], start=True, stop=True)
            un = small.tile([MID, P], bf16, tag="un")
            nc.scalar.activation(un[:], ups[:], AF.Relu)
            o = small.tile([MID, P], bf16, tag="o")
            nc.vector.tensor_mul(o[:], un[:], rsb[:])
            sp["o"] = o

        def emit_e2(ic):
            sc = st[ic]
            # e = exp(cw2 @ h + cb2); rs = exp(-ln(bsum @ e))
            eps = ptp.tile([OUT, P], f32, tag="pt")
            nc.tensor.matmul(out=eps[:], lhsT=ct["cw2t"][:], rhs=sc["h"][:],
                             start=True, stop=True)
            e = small.tile([OUT, P], bf16, tag="e")
            nc.scalar.activation(e[:], eps[:], AF.Exp, bias=ct["cb2"][:])
            sps = ptp.tile([G, P], f32, tag="pt")
            nc.tensor.matmul(out=sps[:], lhsT=ct["bsum"][:], rhs=e[:],
                             start=True, stop=True)
            lns = small.tile([G, P], f32, tag="lns")
            nc.scalar.activation(lns[:], sps[:], AF.Ln)
            rs = small.tile([G, P], bf16, tag="rs")
            nc.scalar.activation(rs[:], lns[:], AF.Exp, scale=-1.0)
            sc["e"], sc["rs"] = e, rs

        def emit_t2(ip):
            # out = wout @ o + x + bout
            sp = st[ip]
            b, t = divmod(ip, nt)
            ops_ = ptp.tile([OUT, P], f32, tag="pt")
            nc.tensor.matmul(out=ops_[:], lhsT=ct["woutt"][:], rhs=sp["o"][:],
                             start=True, stop=True)
            res = small.tile([OUT, P], f32, tag="res")
            nc.vector.scalar_tensor_tensor(
                res[:], ops_[:], ct["bout"][:], sp["xn"][:, 0:P],
                op0=ALU.add, op1=ALU.add)
            nc.sync.dma_start(out_d[b, :, t * P:(t + 1) * P], res[:])
            del st[ip]

        emit_dma(0)
        for i in range(ntot):
            if i + 1 < ntot:
                emit_dma(i + 1)
            emit_s1a(i)
            if i >= 1:
                emit_qpart(i - 1, 0, K // 4)
            emit_s1b(i)
            if i >= 1:
                emit_qpart(i - 1, K // 4, K // 2)
                emit_t1(i - 1)
                emit_t2(i - 1)
            emit_e1(i)
            emit_e2(i)
        emit_qpart(ntot - 1, 0, K // 2)
        emit_t1(ntot - 1)
        emit_t2(ntot - 1)

    return nc


def _legalize_waits(nc):
    """This toolchain's walrus rejects >1 sync-wait per instruction; hoist
    extra waits onto same-engine event-semaphore instructions just before."""
    from concourse import mybir

    n_split = 0
    for fn in nc.m.functions:
        for bb in fn.blocks:
            insts = bb.instructions
            new_list = []
            for inst in insts:
                si = inst.sync_info
                if si is not None and si.on_wait is not None and len(si.on_wait) > 1:
                    waits = list(si.on_wait)
                    for j, w in enumerate(waits[:-1]):
                        ev = mybir.InstEventSemaphore(
                            name=f"{inst.name}-lw{j}", ins=[], outs=[])
                        ev.engine = inst.engine
                        ev.sync_info = mybir.SyncInfo(on_wait=[w], on_update=[])
                        new_list.append(ev)
                        n_split += 1
                    inst.sync_info = mybir.SyncInfo(
                        on_wait=[waits[-1]], on_update=list(si.on_update))
                new_list.append(inst)
            if len(new_list) != len(insts):
                insts[:] = new_list
    return n_split


LAST_RESULTS = None


def kernel(sm_feats, sm_ppfs, w0, b0, w1, b1, w2, b2, w3, b3,
           pw1, pw2, cw1, cw2, cb2, wout, bout):
    global LAST_RESULTS
    import ml_dtypes
    from concourse.bass_utils import run_bass_kernel_spmd

    bf = ml_dtypes.bfloat16
    consts = _build_consts(w0, b0, w1, b1, w2, b2, w3, b3, pw1, pw2,
                           cw1, cw2, cb2, wout, bout)
    nc = _build_program(NS)
    _legalize_waits(nc)

    feats_bf = np.ascontiguousarray(sm_feats).astype(bf)
    ppfs_bf = np.ascontiguousarray(sm_ppfs).astype(bf)
    npix = NS * K

    in_maps = []
    for i in range(NCORES):
        sl = slice(i * NS, (i + 1) * NS)
        m = dict(consts)
        # k-major pixel order within each 256-point tile
        f = feats_bf[:, :, sl, :].reshape(B, CIN, NS // P, P, K)
        m["feats"] = np.ascontiguousarray(
            f.transpose(0, 1, 2, 4, 3)).reshape(B, CIN, npix)
        p = ppfs_bf[:, :, sl, :].reshape(B, PT, NS // P, P, K)
        pp = np.ascontiguousarray(
            p.transpose(0, 1, 2, 4, 3)).reshape(B, PT, npix)
        # pack chunk pairs on partitions: [b, h*8+ch, j*512+s] = pp[b, ch, (2j+h)*512+s]
        v = pp.reshape(B, PT, npix // 1024, 2, FD)
        m["ppfs"] = np.ascontiguousarray(
            v.transpose(0, 3, 1, 2, 4)).reshape(B, 2 * PT, npix // 2)
        in_maps.append(m)

    res = run_bass_kernel_spmd(nc, in_maps, list(range(NCORES)))
    LAST_RESULTS = res
    shards = [res.results[i]["out"] for i in range(NCORES)]
    return np.concatenate(shards, axis=2)


# revision 34
# speedup vs baseline: 1.2689x; 1.0076x over previous
"""Point-Transformer attention block on 8 Trainium2 NeuronCores.

Shards the points axis N across 8 cores (all ops are pointwise in N).
Per core: channels on SBUF partitions, pixels on the free dim.  Pixels are
host-reordered k-major within each 256-point tile, so every (n,k) view
becomes a contiguous slice.  All matmuls use bf16 operands (1 cycle/row)
accumulating in fp32 PSUM.

Fusions vs the reference:
  - x3/d in one psum: stationary [w3; -w2] + [pw2; pw2] -> psum rows
    0:64 = x3_pre (+ptf), 64:128 = d = ptf - x2_pre; one scalar-engine
    activation adds [b3; 0] and materializes both halves as bf16.
  - ppfs host-packed [16, npix/2] (chunk pairs stacked on partitions) so
    one blockdiag(pw1) matmul computes two chunks of r at once.
  - x1 / xfs / cw1 folded:  h = relu( sum_k cw1_k^T @ d_k
                                      + (cw1s @ w1) @ x + cw1s @ (b1 - b2) )
  - softmax denominators applied at the END (U * bcast(1/sum)) so the
    reciprocal never blocks the tensor engine; exp -> 0/1-matmul group
    sums -> fast approx reciprocal -> f32r 0/1-matmul broadcast.
  - k-sum of q: pairwise adds on DVE then 8 accumulating identity matmuls
  - identity skip + bout folded into the final DVE scalar_tensor_tensor.
Emission is software-pipelined: S1(i) then an interleaved zone running
tile i-1's attention-apply against tile i's h/e computation, so neither
the tensor engine nor DVE ever drains.
"""

import numpy as np

B, CIN, N, K = 4, 64, 16384, 16
MID, OUT, PT, SHARE = 64, 128, 8, 8
G = MID // SHARE  # 8 softmax groups
NCORES = 8
NS = N // NCORES          # points per core per batch (2048)
P = 512                   # points per tile
PK = P * K                # pixels per tile (4096)
FD = 512                  # pixel chunk (one matmul's moving width)
NCH = PK // FD            # chunks per tile (8)


def _build_consts(w0, b0, w1, b1, w2, b2, w3, b3, pw1, pw2, cw1, cw2, cb2,
                  wout, bout):
    import ml_dtypes
    f32 = np.float32
    bf = ml_dtypes.bfloat16
    c = {}
    c["w0t"] = np.ascontiguousarray(w0.T).astype(bf)                # [64,128]
    z64 = np.zeros((OUT, MID), np.float32)
    c["w3lo"] = np.concatenate([w3.T, z64], axis=1).astype(bf)      # [128,128]
    c["w3hi"] = np.concatenate([z64, w3.T], axis=1).astype(bf)      # [128,128]
    c["mw2lo"] = np.concatenate([-w2.T, z64], axis=1).astype(bf)    # [128,128]
    c["mw2hi"] = np.concatenate([z64, -w2.T], axis=1).astype(bf)    # [128,128]
    zpw = np.zeros_like(pw2.T)
    c["pw2blk"] = np.block([[pw2.T, zpw], [zpw, pw2.T]]).astype(bf)  # [128,128]
    pw1t = np.ascontiguousarray(pw1.T)                              # [8,64]
    pw1t2 = np.zeros((2 * PT, OUT), np.float32)
    pw1t2[0:PT, 0:MID] = pw1t
    pw1t2[PT:2 * PT, MID:OUT] = pw1t
    c["pw1t2"] = pw1t2.astype(bf)                                   # [16,128]
    cw1r = cw1.reshape(G, MID, K)
    cw1s = cw1r.sum(-1)                                             # [8,64]
    c["cat"] = np.ascontiguousarray((cw1s @ w1).T).astype(bf)       # [128,8]
    gkp = np.zeros((OUT, (K // 2) * G), np.float32)
    for j in range(K // 2):
        gkp[0:MID, j * G:(j + 1) * G] = cw1r[:, :, 2 * j].T
        gkp[MID:OUT, j * G:(j + 1) * G] = cw1r[:, :, 2 * j + 1].T
    c["gkp"] = gkp.astype(bf)                                       # [128,64]
    c["hb"] = np.ascontiguousarray((cw1s @ (b1 - b2))[:, None], f32)  # [8,1]
    c["cw2t"] = np.ascontiguousarray(cw2.T).astype(bf)              # [8,128]
    c["cb2"] = np.ascontiguousarray(cb2[:, None], f32)              # [128,1]
    bsum = np.zeros((OUT, G), f32)
    for g in range(G):
        bsum[g * K:(g + 1) * K, g] = 1.0
    c["bsum"] = bsum.astype(bf)                                     # [128,8]
    bsum64 = np.zeros((G, MID), f32)
    for m in range(MID):
        bsum64[m % G, m] = 1.0
    c["bsum64"] = bsum64.astype(bf)                                 # [8,64]
    bk2 = np.zeros((OUT, (K // 2) * OUT), f32)
    for j in range(K // 2):
        for m in range(OUT):
            k = 2 * j + (m >= MID)
            bk2[(m % G) * K + k, j * OUT + m] = 1.0
    c["bk2"] = bk2.astype(bf)                                       # [128,1024]
    c["woutt"] = np.ascontiguousarray(wout.T).astype(bf)            # [64,128]
    c["eye2"] = np.concatenate(
        [np.eye(MID, dtype=f32)] * 2, axis=0).astype(bf)            # [128,64]
    c["b0"] = np.ascontiguousarray(b0[:, None], f32)                # [128,1]
    c["b3b"] = np.concatenate([b3, b3])[:, None].astype(f32)        # [128,1]
    c["bout"] = np.ascontiguousarray(bout[:, None], f32)            # [128,1]
    return c


def _build_program(ns_pts):
    import concourse.bass as bass
    import concourse.tile as tile
    from concourse import mybir
    from contextlib import ExitStack

    f32 = mybir.dt.float32
    f32r = mybir.dt.float32r
    bf16 = mybir.dt.bfloat16
    AF = mybir.ActivationFunctionType
    ALU = mybir.AluOpType
    nt = ns_pts // P

    nc = bass.Bass()
    feats_d = nc.declare_dram_parameter("feats", [B, CIN, ns_pts * K], bf16, isOutput=False)
    ppfs_d = nc.declare_dram_parameter("ppfs", [B, 2 * PT, ns_pts * K // 2], bf16, isOutput=False)
    cshape = dict(
        w0t=([CIN, OUT], bf16), w3lo=([OUT, OUT], bf16), w3hi=([OUT, OUT], bf16),
        mw2lo=([OUT, OUT], bf16), mw2hi=([OUT, OUT], bf16), pw2blk=([OUT, OUT], bf16),
        pw1t2=([2 * PT, OUT], bf16), cat=([OUT, G], bf16),
        gkp=([OUT, (K // 2) * G], bf16),
        hb=([G, 1], f32), cw2t=([G, OUT], bf16), cb2=([OUT, 1], f32),
        bsum=([OUT, G], bf16), bsum64=([G, MID], bf16),
        bk2=([OUT, (K // 2) * OUT], bf16),
        woutt=([MID, OUT], bf16), eye2=([OUT, MID], bf16),
        b0=([OUT, 1], f32), b3b=([OUT, 1], f32), bout=([OUT, 1], f32),
    )
    cdram = {k: nc.declare_dram_parameter(k, v[0], v[1], isOutput=False)
             for k, v in cshape.items()}
    out_d = nc.declare_dram_parameter("out", [B, OUT, ns_pts], f32, isOutput=True)

    ntot = B * nt

    with tile.TileContext(nc) as tc, ExitStack() as ctx:
        consts = ctx.enter_context(tc.tile_pool(name="consts", bufs=1))
        ct = {k: consts.tile_from(v[:], name=k) for k, v in cdram.items()}

        io = ctx.enter_context(tc.tile_pool(name="io", bufs=2))
        sb = ctx.enter_context(tc.tile_pool(name="sb", bufs=2))
        small = ctx.enter_context(tc.tile_pool(name="small", bufs=2))
        pix = ctx.enter_context(tc.tile_pool(name="pix", bufs=2, space="PSUM"))
        ptp = ctx.enter_context(tc.tile_pool(name="ptp", bufs=2, space="PSUM"))
        wfp = ctx.enter_context(tc.tile_pool(name="wfp", bufs=2, space="PSUM"))

        st = {}  # per-tile live handles

        def emit_dma(i):
            b, t = divmod(i, nt)
            px = slice(t * PK, (t + 1) * PK)
            px2 = slice(t * PK // 2, (t + 1) * PK // 2)
            ft = io.tile([CIN, PK], bf16, tag="ft")
            if i == 0:
                # column-split the cold-start load so chunk 0's matmul can
                # begin after the first quarter lands
                for w in range(4):
                    cs = slice(w * PK // 4, (w + 1) * PK // 4)
                    nc.sync.dma_start(ft[:, cs],
                                      feats_d[b, :, t * PK + w * PK // 4:
                                              t * PK + (w + 1) * PK // 4])
            else:
                nc.sync.dma_start(ft[0:CIN // 2, :], feats_d[b, 0:CIN // 2, px])
                nc.sync.dma_start(ft[CIN // 2:CIN, :], feats_d[b, CIN // 2:CIN, px])
            pf = io.tile([2 * PT, PK // 2], bf16, tag="pf")
            nc.sync.dma_start(pf[:], ppfs_d[b, :, px2])
            st[i] = dict(ft=ft, pf=pf)

        def emit_s1a(i):
            s = st[i]
            ft, pf = s["ft"], s["pf"]
            # xn = relu(w0 @ feats + b0)            [128, PK] bf16
            xn = sb.tile([OUT, PK], bf16, tag="xn")
            s["xn"] = xn
            for u in range(NCH // 2):
                ps = pix.tile([OUT, 2 * FD], f32, tag="pix")
                for h in range(2):
                    cs = slice((2 * u + h) * FD, (2 * u + h + 1) * FD)
                    nc.tensor.matmul(out=ps[:, h * FD:(h + 1) * FD],
                                     lhsT=ct["w0t"][:], rhs=ft[:, cs],
                                     start=True, stop=True)
                if u < 5:
                    nc.scalar.activation(xn[:, 2 * u * FD:(2 * u + 2) * FD],
                                         ps[:], AF.Relu, bias=ct["b0"][:])
                else:
                    nc.vector.tensor_scalar(
                        xn[:, 2 * u * FD:(2 * u + 2) * FD], ps[:],
                        ct["b0"][:], 0.0, op0=ALU.add, op1=ALU.max)
            # r = relu(pw1 @ ppfs), chunk c on rows 64*(c%2), cols (c//2)*FD
            r = sb.tile([OUT, PK // 2], bf16, tag="r")
            s["r"] = r
            for u in range(NCH // 4):
                ps = pix.tile([OUT, 2 * FD], f32, tag="pix")
                for h in range(2):
                    js = slice((2 * u + h) * FD, (2 * u + h + 1) * FD)
                    nc.tensor.matmul(out=ps[:, h * FD:(h + 1) * FD],
                                     lhsT=ct["pw1t2"][:], rhs=pf[:, js],
                                     start=True, stop=True)
                if u < NCH // 4 - 2:
                    nc.scalar.activation(r[:, 2 * u * FD:(2 * u + 2) * FD],
                                         ps[:], AF.Relu)
                elif u < NCH // 4 - 1:
                    nc.vector.tensor_scalar_max(
                        r[:, 2 * u * FD:(2 * u + 2) * FD], ps[:], 0.0)
                else:
                    s["rps_last"] = (ps, 2 * u * FD)

        def emit_s1b(i):
            s = st[i]
            xn, r = s["xn"], s["r"]
            ps_last, off = s.pop("rps_last")
            nc.vector.tensor_scalar_max(r[:, off:off + 2 * FD], ps_last[:], 0.0)
            # pair-stacked psums: [x3_2j ; x3_2j+1] and [d_2j ; d_2j+1]
            x3p = sb.tile([OUT, PK // 2], bf16, tag="x3p")
            dp = sb.tile([OUT, PK // 2], bf16, tag="dp")
            nsc = 0
            for v in range(K // 4):
                for which in range(2):  # 0: x3 pairs, 1: d pairs
                    ps = pix.tile([OUT, 2 * FD], f32, tag="pix")
                    for w in range(2):
                        j = 2 * v + w
                        hs = slice(w * FD, (w + 1) * FD)
                        lo = "w3lo" if which == 0 else "mw2lo"
                        hi = "w3hi" if which == 0 else "mw2hi"
                        nc.tensor.matmul(out=ps[:, hs], lhsT=ct[lo][:],
                                         rhs=xn[:, (2 * j) * FD:(2 * j + 1) * FD],
                                         start=True, stop=False)
                        nc.tensor.matmul(out=ps[:, hs], lhsT=ct[hi][:],
                                         rhs=xn[:, (2 * j + 1) * FD:(2 * j + 2) * FD],
                                         start=False, stop=False)
                        nc.tensor.matmul(out=ps[:, hs], lhsT=ct["pw2blk"][:],
                                         rhs=r[:, j * FD:(j + 1) * FD],
                                         start=False, stop=True)
                    dst = (x3p if which == 0 else dp)[:, 2 * v * FD:(2 * v + 2) * FD]
                    if which == 0:
                        if nsc < 3:
                            nc.scalar.activation(dst, ps[:], AF.Identity,
                                                 bias=ct["b3b"][:])
                        else:
                            nc.vector.tensor_scalar_add(dst, ps[:], ct["b3b"][:])
                        nsc += 1
                    else:
                        if v < 2:
                            nc.scalar.activation(dst, ps[:], AF.Identity)
                        else:
                            nc.vector.tensor_copy(dst, ps[:])
            s["x3p"], s["dp"] = x3p, dp

        def emit_qpart(ip, jlo, jhi):
            sp = st[ip]
            if "q" not in sp:
                sp["q"] = sb.tile([OUT, PK // 2], bf16, tag="q", name="q")
            q = sp["q"]
            for j in range(jlo, jhi):
                wf = wfp.tile([OUT, P], f32, tag="wf")
                nc.tensor.matmul(out=wf[:],
                                 lhsT=ct["bk2"][:, j * OUT:(j + 1) * OUT],
                                 rhs=sp["e"][:], start=True, stop=True)
                nc.vector.tensor_mul(q[:, j * P:(j + 1) * P],
                                     sp["x3p"][:, j * P:(j + 1) * P], wf[:])

        def emit_e1(ic):
            sc = st[ic]
            # h = relu(sum_k cw1_k^T @ d_k + Ca @ x + hb)   [8, P]
            hps = ptp.tile([G, P], f32, tag="pt")
            for j in range(K // 2):
                nc.tensor.matmul(out=hps[:], lhsT=ct["gkp"][:, j * G:(j + 1) * G],
                                 rhs=sc["dp"][:, j * P:(j + 1) * P],
                                 start=(j == 0), stop=False)
            nc.tensor.matmul(out=hps[:], lhsT=ct["cat"][:],
                             rhs=sc["xn"][:, 0:P], start=False, stop=True)
            h = small.tile([G, P], bf16, tag="h")
            nc.scalar.activation(h[:], hps[:], AF.Relu, bias=ct["hb"][:])
            sc["h"] = h

        def emit_t1(ip):
            # U = sum_j q2_j; o = relu(U) * bcast(rs)
            sp = st[ip]
            q = sp["q"]
            ups = ptp.tile([MID, P], f32, tag="pt")
            for j in range(K // 2):
                nc.tensor.matmul(out=ups[:], lhsT=ct["eye2"][:],
                                 rhs=q[:, j * P:(j + 1) * P],
                                 start=(j == 0), stop=(j == K // 2 - 1))
            rsb = ptp.tile([MID, P], f32, tag="pt")
            nc.tensor.matmul(out=rsb[:], lhsT=ct["bsum64"][:],
                             rhs=sp["rs"][:], start=True, stop=True)
            un = small.tile([MID, P], bf16, tag="un")
            nc.scalar.activation(un[:], ups[:], AF.Relu)
            o = small.tile([MID, P], bf16, tag="o")
            nc.vector.tensor_mul(o[:], un[:], rsb[:])
            sp["o"] = o

        def emit_e2(ic):
            sc = st[ic]
            # e = exp(cw2 @ h + cb2); rs = exp(-ln(bsum @ e))
            eps = ptp.tile([OUT, P], f32, tag="pt")
            nc.tensor.matmul(out=eps[:], lhsT=ct["cw2t"][:], rhs=sc["h"][:],
                             start=True, stop=True)
            e = small.tile([OUT, P], bf16, tag="e")
            nc.scalar.activation(e[:], eps[:], AF.Exp, bias=ct["cb2"][:])
            sps = ptp.tile([G, P], f32, tag="pt")
            nc.tensor.matmul(out=sps[:], lhsT=ct["bsum"][:], rhs=e[:],
                             start=True, stop=True)
            lns = small.tile([G, P], f32, tag="lns")
            nc.scalar.activation(lns[:], sps[:], AF.Ln)
            rs = small.tile([G, P], bf16, tag="rs")
            nc.scalar.activation(rs[:], lns[:], AF.Exp, scale=-1.0)
            sc["e"], sc["rs"] = e, rs

        def emit_t2(ip):
            # out = wout @ o + x + bout
            sp = st[ip]
            b, t = divmod(ip, nt)
            ops_ = ptp.tile([OUT, P], f32, tag="pt")
            nc.tensor.matmul(out=ops_[:], lhsT=ct["woutt"][:], rhs=sp["o"][:],
                             start=True, stop=True)
            res = small.tile([OUT, P], f32, tag="res")
            nc.vector.scalar_tensor_tensor(
                res[:], ops_[:], ct["bout"][:], sp["xn"][:, 0:P],
                op0=ALU.add, op1=ALU.add)
            nc.sync.dma_start(out_d[b, :, t * P:(t + 1) * P], res[:])
            del st[ip]

        emit_dma(0)
        for i in range(ntot):
            if i + 1 < ntot:
                emit_dma(i + 1)
            emit_s1a(i)
            if i >= 1:
                emit_qpart(i - 1, 0, K // 4)
            emit_s1b(i)
            if i >= 1:
                emit_qpart(i - 1, K // 4, K // 2)
                emit_t1(i - 1)
                emit_t2(i - 1)
            emit_e1(i)
            emit_e2(i)
        emit_qpart(ntot - 1, 0, K // 2)
        emit_t1(ntot - 1)
        emit_t2(ntot - 1)

    return nc


def _legalize_waits(nc):
    """This toolchain's walrus rejects >1 sync-wait per instruction; hoist
    extra waits onto same-engine event-semaphore instructions just before."""
    from concourse import mybir

    n_split = 0
    for fn in nc.m.functions:
        for bb in fn.blocks:
            insts = bb.instructions
            new_list = []
            for inst in insts:
                si = inst.sync_info
                if si is not None and si.on_wait is not None and len(si.on_wait) > 1:
                    waits = list(si.on_wait)
                    for j, w in enumerate(waits[:-1]):
                        ev = mybir.InstEventSemaphore(
                            name=f"{inst.name}-lw{j}", ins=[], outs=[])
                        ev.engine = inst.engine
                        ev.sync_info = mybir.SyncInfo(on_wait=[w], on_update=[])
                        new_list.append(ev)
                        n_split += 1
                    inst.sync_info = mybir.SyncInfo(
                        on_wait=[waits[-1]], on_update=list(si.on_update))
                new_list.append(inst)
            if len(new_list) != len(insts):
                insts[:] = new_list
    return n_split


LAST_RESULTS = None


def kernel(sm_feats, sm_ppfs, w0, b0, w1, b1, w2, b2, w3, b3,
           pw1, pw2, cw1, cw2, cb2, wout, bout):
    global LAST_RESULTS
    import ml_dtypes
    from concourse.bass_utils import run_bass_kernel_spmd

    bf = ml_dtypes.bfloat16
    consts = _build_consts(w0, b0, w1, b1, w2, b2, w3, b3, pw1, pw2,
                           cw1, cw2, cb2, wout, bout)
    nc = _build_program(NS)
    _legalize_waits(nc)

    feats_bf = np.ascontiguousarray(sm_feats).astype(bf)
    ppfs_bf = np.ascontiguousarray(sm_ppfs).astype(bf)
    npix = NS * K

    in_maps = []
    for i in range(NCORES):
        sl = slice(i * NS, (i + 1) * NS)
        m = dict(consts)
        # k-major pixel order within each 256-point tile
        f = feats_bf[:, :, sl, :].reshape(B, CIN, NS // P, P, K)
        m["feats"] = np.ascontiguousarray(
            f.transpose(0, 1, 2, 4, 3)).reshape(B, CIN, npix)
        p = ppfs_bf[:, :, sl, :].reshape(B, PT, NS // P, P, K)
        pp = np.ascontiguousarray(
            p.transpose(0, 1, 2, 4, 3)).reshape(B, PT, npix)
        # pack chunk pairs on partitions: [b, h*8+ch, j*512+s] = pp[b, ch, (2j+h)*512+s]
        v = pp.reshape(B, PT, npix // 1024, 2, FD)
        m["ppfs"] = np.ascontiguousarray(
            v.transpose(0, 3, 1, 2, 4)).reshape(B, 2 * PT, npix // 2)
        in_maps.append(m)

    res = run_bass_kernel_spmd(nc, in_maps, list(range(NCORES)))
    LAST_RESULTS = res
    shards = [res.results[i]["out"] for i in range(NCORES)]
    return np.concatenate(shards, axis=2)
